# revision 46
# baseline (speedup 1.0000x reference)
"""AdaptiveConv2DMod kernel for 8 TRN2 NeuronCores.

Data-parallel over batch: B=16 -> 2 samples per core, base weights replicated.
Per sample: softmax-mix 4 base kernels, modulate by (1+mod) over input
channels, demodulate per output channel, then 3x3 same-conv.

Conv is computed as 9 shifted matmuls (x2 input-channel chunks) accumulated
in PSUM, bf16 compute / fp32 accumulate. fmap/weights are cast to bf16 on
the host (the kernel computed in bf16 anyway) to halve input DMA; softmax over
the 4 kernel logits is computed on the host; output is written fp32 from
the demod-scaling psum drain.

Row-tile groups of 4: within a group the taps are outer and the row tiles
inner, so the stationary PE weights are reused 4x (deduped Ldweights) while
group drains still overlap the next group's matmuls.
"""

from contextlib import ExitStack

import numpy as np

import concourse.bass as bass
import concourse.mybir as mybir
import concourse.tile as tile
from concourse import bacc
from concourse.bass_utils import run_bass_kernel_spmd

F32 = mybir.dt.float32
BF16 = mybir.dt.bfloat16
NP_BF16 = mybir.dt.np(BF16)

N_CORES = 8
B_LOC = 2          # samples per core
C = 256            # input channels (I)
O = 256            # output channels
H = W = 64
K = 3
NK = 4             # num base kernels
CI = 2             # input channel chunks of 128
CO = 2             # output channel chunks of 128
NT = 8             # row tiles (8 rows x 64 cols = 512 free)
GRP = 4            # row tiles per drain group
ROWS_PER_NT = H // NT


def _build_nc(repeat=1, loop_n=0, parts="full"):
    nc = bacc.Bacc("TRN2", target_bir_lowering=False, debug=False,
                   num_devices=N_CORES)
    fmap = nc.declare_dram_parameter("fmap", [B_LOC, C, H, W], BF16,
                                     isOutput=False)
    mod = nc.declare_dram_parameter("mod", [B_LOC, C], F32, isOutput=False)
    # softmax(kernel_mod) is computed on the host (tiny [B,4] op) so the
    # device pipeline has no ACT dependency before the first weight mix
    kmod = nc.declare_dram_parameter("attn_in", [B_LOC, NK], F32,
                                     isOutput=False)
    # weights are host-permuted to [n, o, ci, ky, kx, i128] so the on-chip
    # mix pipeline is (kl, i)-ordered with packed last dims throughout (DVE
    # 2x mode), the per-tap transpose input needs no strided rearrange, and
    # each per-(n, co, ci) tile is one DMA of contiguous 2.3KB runs
    weights = nc.declare_dram_parameter("weights", [NK, O, CI, K, K, 128],
                                        BF16, isOutput=False)
    out = nc.declare_dram_parameter("out", [B_LOC, O, H, W], F32,
                                    isOutput=True)

    with ExitStack() as ctx:
        tc = ctx.enter_context(tile.TileContext(nc))
        pools = _make_pools(ctx, tc)
        if loop_n:
            with tc.For_i(0, loop_n, 1):
                _build_body(tc, pools, fmap.ap(), mod.ap(), kmod.ap(),
                            weights.ap(), out.ap(), parts, warmup=False)
        else:
            for r in range(repeat):
                _build_body(tc, pools, fmap.ap(), mod.ap(), kmod.ap(),
                            weights.ap(), out.ap(), parts, warmup=(r == 0))
    _dedupe_ldweights(nc)
    nc.compile()
    return nc


def _dedupe_ldweights(nc):
    """Remove PE weight reloads that are byte-identical to the previous
    Ldweights and carry no semaphore waits/updates (the split emits one
    Ldweights per matmul even when the stationary operand is unchanged)."""
    removed = 0
    pe = mybir.EngineType.PE
    for blk in nc.main_func.blocks:
        last_key = None
        keep = []
        for inst in blk.instructions:
            tn = type(inst).__name__
            eng = getattr(inst, "engine", None)
            if tn == "InstLdweights":
                key = repr(inst.ins)
                if (key == last_key and inst.sync_info is None):
                    removed += 1
                    continue
                last_key = key
            elif tn == "InstMatmult":
                pass
            elif eng == pe:
                last_key = None
            keep.append(inst)
        blk.instructions[:] = keep
    return removed


def _make_pools(ctx, tc):
    return {
        "const": ctx.enter_context(tc.tile_pool(name="const", bufs=2)),
        "wnat": ctx.enter_context(tc.tile_pool(name="wnat", bufs=3)),
        "mix": ctx.enter_context(tc.tile_pool(name="mix", bufs=4)),
        "wt": ctx.enter_context(tc.tile_pool(name="wt", bufs=B_LOC * CO)),
        "fm": ctx.enter_context(tc.tile_pool(name="fm", bufs=4)),
        "outp": ctx.enter_context(tc.tile_pool(name="outp", bufs=2)),
        "small": ctx.enter_context(tc.tile_pool(name="small", bufs=12)),
        "psconv": ctx.enter_context(
            tc.tile_pool(name="psconv", bufs=8, space="PSUM")),
    }


def _build_body(tc, pools, fmap, mod, kmod, weights, out, parts="full",
                warmup=True):
    nc = tc.nc

    if parts == "empty":
        t = pools["const"].tile([128, 1], F32, tag="emp")
        nc.vector.memset(t[:], 0.0)
        return

    const = pools["const"]
    wnatp = pools["wnat"]
    mixp = pools["mix"]
    wtp = pools["wt"]
    fmp = pools["fm"]
    outp = pools["outp"]
    smallp = pools["small"]
    psconv = pools["psconv"]

    # ---- small inputs (sync queue) -----------------------------------------
    attn = const.tile([128, B_LOC, NK], F32)
    nc.sync.dma_start(out=attn[:], in_=kmod[None, :, :].broadcast_to(
        [128, B_LOC, NK]))
    # mod lands as one tiny single-partition DMA; broadcast on-chip (the
    # 128-partition broadcast DMA would sit on the startup-critical DMA path)
    m_sm = const.tile([1, B_LOC, C], F32)
    nc.sync.dma_start(out=m_sm[:], in_=mod[None, :, :])
    m_bc = const.tile([128, B_LOC, C], F32)

    eps = const.tile([128, 1], F32)
    nc.vector.memset(eps[:], 1e-8)

    # PE warmup: dummy matmuls during the startup DMA window so the HAM
    # clock gate is at 8/8 (2.4 GHz) when the real conv stream begins (only
    # for the first body — loop iterations are already warm)
    if warmup:
        warm = const.tile([128, 640], BF16)
        nc.vector.memset(warm[:], 0.125)
        wps = psconv.tile([128, GRP, ROWS_PER_NT, W], F32, tag="psg", bufs=2,
                          name="warm_ps")
        wps_flat = wps.rearrange("p g r w -> p (g r w)")
        for i in range(20):
            nc.tensor.matmul(wps_flat[:, 0:512], warm[:, 0:128],
                             warm[:, 128:640], start=(i == 0), stop=(i == 19))

    # preload the Square/Sqrt activation tables off the critical path (the
    # auto-inserted LoadActFuncSet otherwise serializes before the first
    # demod Square)
    preld = const.tile([128, 1], F32)
    nc.scalar.activation(preld[:], eps[:], mybir.ActivationFunctionType.Square)
    nc.scalar.activation(preld[:], eps[:], mybir.ActivationFunctionType.Sqrt,
                         bias=eps[:])
    # the (1 + mod) add is emitted lazily just before its first use so the
    # in-order DVE queue isn't blocked on the m_bc DMA before the first mix
    m_ready = []

    def ensure_m():
        if not m_ready:
            nc.gpsimd.partition_broadcast(m_bc[:], m_sm[0:1, :, :])
            nc.vector.tensor_scalar_add(m_bc[:], m_bc[:], 1.0)  # 1 + mod
            m_ready.append(True)

    # ---- input DMAs: all on the Pool queue, in first-use priority order ----
    # (single queue => transfer order == issue order, so fmaps can't steal
    # DMA bandwidth from the startup-critical weight chunks)
    # w9[co][ci]: [128(o), NK, 9(kl), 128(i)] bf16, one DMA each with
    # contiguous 2.3KB runs
    w9 = [[None] * CI for _ in range(CO)]
    fm_raw = [[None] * CI for _ in range(B_LOC)]

    def load_w(co, ci, split=False):
        t = wnatp.tile([128, NK, K * K, 128], BF16, tag=f"wn{co}{ci}",
                       bufs=1, name=f"w9_{co}_{ci}")
        if split:
            # per-n DMAs for the startup-critical first chunk, so the first
            # mix mul can begin as soon as base kernel 0 lands
            for n in range(NK):
                nc.gpsimd.dma_start(
                    out=t[:, n, :, :],
                    in_=weights[n, co * 128:(co + 1) * 128, ci, :, :, :]
                    .rearrange("o ky kx i -> o (ky kx) i"))
        else:
            nc.gpsimd.dma_start(
                out=t[:],
                in_=weights[:, co * 128:(co + 1) * 128, ci, :, :, :].rearrange(
                    "n o ky kx i -> o n (ky kx) i"))
        w9[co][ci] = t

    def w9_slice(n, co, ci):
        return w9[co][ci][:, n, :, :]

    def load_fmap(b, ci):
        raw = fmp.tile([128, H, W], BF16, tag="fmraw", name=f"fmraw{b}_{ci}")
        nc.gpsimd.dma_start(
            out=raw[:], in_=fmap[b, ci * 128:(ci + 1) * 128, :, :])
        fm_raw[b][ci] = raw

    # NOTE: load emission is interleaved with the weight-pipe blocks below —
    # Tile chains each DMA-family instruction to the completion of the one
    # emitted just before it, so a transpose must not be preceded by a DMA
    # it doesn't actually need.

    # ---- per-sample weight pipeline ----------------------------------------
    # w_T[b][co]: [128(i in chunk), (ci,kl)=18, o-chunk=128] bf16 modulated
    # transposed weights; one xbar transpose per (b, co, ci) half.
    w_T = [[None] * CO for _ in range(B_LOC)]
    dscale = [[None] * CO for _ in range(B_LOC)]
    den_h = [[[None] * CI for _ in range(CO)] for _ in range(B_LOC)]

    wmods = [[[None] * CI for _ in range(CO)] for _ in range(B_LOC)]

    def mix_block(b, co, ci, transposes=True):
        """mix 4 base kernels + modulate by (1+mod) -> wmod (kl, i)-ordered."""
        wn = [w9_slice(n, co, ci) for n in range(NK)]
        t0 = mixp.tile([128, K * K, 128], BF16, tag="mixa")
        t1 = mixp.tile([128, K * K, 128], BF16, tag="mixb")
        nc.vector.tensor_scalar_mul(t0[:], wn[0], attn[:, b, 0:1])
        nc.vector.tensor_scalar_mul(t1[:], wn[1], attn[:, b, 1:2])
        nc.vector.tensor_add(t0[:], t0[:], t1[:])
        nc.vector.tensor_scalar_mul(t1[:], wn[2], attn[:, b, 2:3])
        nc.vector.tensor_add(t0[:], t0[:], t1[:])
        nc.vector.tensor_scalar_mul(t1[:], wn[3], attn[:, b, 3:4])
        nc.vector.tensor_add(t0[:], t0[:], t1[:])
        ensure_m()
        wmod = mixp.tile([128, K * K, 128], BF16, tag="wmod", bufs=4)
        nc.vector.tensor_mul(
            wmod[:], t0[:],
            m_bc[:, b, None, ci * 128:(ci + 1) * 128].broadcast_to(
                [128, K * K, 128]))
        wmods[b][co][ci] = wmod
        # transpose fires as soon as this half's wmod is ready (it gates the
        # conv); the demod scale is applied at psum-drain time instead
        if transposes:
            wt = w_T[b][co]
            nc.sync.dma_start(out=wt[:, ci * K * K:(ci + 1) * K * K, :],
                              in_=wmod[:], transpose=True)
        # demod denominator half: sum over free dims of wmod^2 (per o-part)
        sqscratch = mixp.tile([128, K * K, 128], BF16, tag="sqs", bufs=2)
        dh = smallp.tile([128, 1], F32, tag="den", name=f"den{b}_{co}_{ci}")
        nc.scalar.activation(
            sqscratch[:], wmod[:],
            mybir.ActivationFunctionType.Square, accum_out=dh[:])
        den_h[b][co][ci] = dh

    def finalize_block(b, co):
        ds = smallp.tile([128, 1], F32, tag="dsc")
        nc.vector.tensor_add(ds[:], den_h[b][co][0][:], den_h[b][co][1][:])
        nc.scalar.activation(ds[:], ds[:],
                             mybir.ActivationFunctionType.Sqrt, bias=eps[:])
        nc.vector.reciprocal(ds[:], ds[:])
        dscale[b][co] = ds

    def pipe(b, co, transposes=True):
        wt = wtp.tile([128, K * K * CI, 128], BF16, tag="wt",
                      name=f"wT{b}_{co}")
        w_T[b][co] = wt
        if not transposes:
            nc.vector.memset(wt[:], 0.25)
        for ci in range(CI):
            mix_block(b, co, ci, transposes)
        finalize_block(b, co)

    if parts == "conv":
        for b in range(B_LOC):
            for ci in range(CI):
                load_fmap(b, ci)
            for co in range(CO):
                wt = wtp.tile([128, K * K * CI, 128], BF16, tag="wt",
                              name=f"wTd{b}_{co}")
                nc.vector.memset(wt[:], 0.25)
                w_T[b][co] = wt
                ds = smallp.tile([128, 1], F32, tag="dsc")
                nc.vector.memset(ds[:], 1.0)
                dscale[b][co] = ds
    else:
        tr = parts != "wnotr"
        load_w(0, 0, split=True)
        load_fmap(0, 0)
        load_w(0, 1)
        pipe(0, 0, tr)
        load_fmap(0, 1)
        load_w(1, 0)
        load_w(1, 1)
        pipe(0, 1, tr)
        load_fmap(1, 0)
        load_fmap(1, 1)
        pipe(1, 0, tr)
        pipe(1, 1, tr)

    # ---- conv: out[o, y, x] += sum_{ci,ky,kx} w.T @ fmap_shifted -----------
    # Row-tile groups of GRP. Within a group: ci -> tap -> row-tile, so the
    # stationary weights are constant across the GRP inner matmuls (deduped
    # Ldweights) and each group's psum drains overlap the next group.
    # Boundary taps use column/row-clamped access patterns; elements a tap
    # skips are covered by other taps' writes (per-element has_written).
    def conv(b, co):
        for g in range(NT // GRP):
            # one 4-bank psum tile per group; each matmul writes a 1-bank
            # slice; the whole group DMAs straight to DRAM (weights carry
            # the demod scale already)
            ps = psconv.tile([128, GRP, ROWS_PER_NT, W], F32, tag="psg",
                             bufs=2, name=f"ps{b}_{co}_{g}")
            for ci in range(CI):
                for ky in range(K):
                    for kx in range(K):
                        kl = ky * K + kx
                        lhsT = w_T[b][co][:, ci * K * K + kl, :]
                        x0 = max(0, kx - 1)
                        xo = max(0, 1 - kx)
                        wn = W - abs(kx - 1)
                        for j in range(GRP):
                            nt = g * GRP + j
                            r0 = nt * ROWS_PER_NT + ky - 1
                            ny = ROWS_PER_NT
                            rskip = 0
                            if r0 < 0:                # clamp top
                                r0, ny, rskip = 0, ROWS_PER_NT - 1, 1
                            if r0 + ny > H:           # clamp bottom
                                ny = H - r0
                            rhs = fm_raw[b][ci][:, r0:r0 + ny, x0:x0 + wn]
                            nc.tensor.matmul(
                                ps[:, j, rskip:rskip + ny, xo:xo + wn],
                                lhsT, rhs,
                                start=(ci == 0 and kl == 0),
                                stop=(ci == CI - 1 and kl == K * K - 1))
            # drain: demod scale applied here (per-o-partition), fp32 out via
            # the ACT HWDGE queue so the Pool queue stays free for input DMAs.
            # The very last group drains in two halves to shorten the tail.
            ot = outp.tile([128, GRP * ROWS_PER_NT, W], F32, tag="ot", bufs=2)
            rows = GRP * ROWS_PER_NT
            y0 = g * rows
            halves = 2 if (b == B_LOC - 1 and co == CO - 1
                           and g == NT // GRP - 1) else 1
            hr = rows // halves
            for h in range(halves):
                nc.scalar.mul(
                    ot[:, h * hr:(h + 1) * hr, :],
                    ps[:, h * GRP // halves:(h + 1) * GRP // halves, :, :],
                    dscale[b][co][:])
                nc.scalar.dma_start(
                    out=out[b, co * 128:(co + 1) * 128,
                            y0 + h * hr:y0 + (h + 1) * hr, :],
                    in_=ot[:, h * hr:(h + 1) * hr, :])

    if parts not in ("wpipe", "wnotr"):
        for b in range(B_LOC):
            for co in range(CO):
                conv(b, co)


_NC_CACHE = {}


def _get_nc(repeat=1, loop_n=0, parts="full"):
    key = (repeat, loop_n, parts)
    if key not in _NC_CACHE:
        _NC_CACHE[key] = _build_nc(repeat, loop_n, parts)
    return _NC_CACHE[key]


def _make_in_maps(fmap, mod, kernel_mod, weights):
    fmap_bf = np.ascontiguousarray(fmap.astype(NP_BF16))
    # [n, o, i, ky, kx] -> [n, o, ci, ky, kx, i128] (see DRAM declaration)
    weights_bf = np.ascontiguousarray(
        weights.astype(NP_BF16)
        .reshape(NK, O, CI, 128, K, K)
        .transpose(0, 1, 2, 4, 5, 3))
    # host-side softmax over the 4 kernel logits (tiny)
    e = np.exp(kernel_mod.astype(np.float64)
               - kernel_mod.max(axis=-1, keepdims=True))
    attn = (e / e.sum(axis=-1, keepdims=True)).astype(np.float32)
    in_maps = []
    for c in range(N_CORES):
        s = slice(c * B_LOC, (c + 1) * B_LOC)
        in_maps.append({
            "fmap": np.ascontiguousarray(fmap_bf[s]),
            "mod": np.ascontiguousarray(mod[s]),
            "attn_in": np.ascontiguousarray(attn[s]),
            "weights": weights_bf,
        })
    return in_maps


def kernel(fmap, mod, kernel_mod, weights, _trace=False):
    fmap = np.asarray(fmap, dtype=np.float32)
    mod = np.asarray(mod, dtype=np.float32)
    kernel_mod = np.asarray(kernel_mod, dtype=np.float32)
    weights = np.ascontiguousarray(np.asarray(weights, dtype=np.float32))

    nc = _get_nc()
    in_maps = _make_in_maps(fmap, mod, kernel_mod, weights)
    res = run_bass_kernel_spmd(nc, in_maps, list(range(N_CORES)), trace=_trace)
    outs = np.concatenate(
        [res.results[c]["out"].astype(np.float32) for c in range(N_CORES)],
        axis=0)
    if _trace:
        kernel.last_results = res
    return outs


# revision 47
# speedup vs baseline: 1.0407x; 1.0407x over previous
"""AdaptiveConv2DMod kernel for 8 TRN2 NeuronCores.

Data-parallel over batch: B=16 -> 2 samples per core, base weights replicated.
Per sample: softmax-mix 4 base kernels, modulate by (1+mod) over input
channels, demodulate per output channel, then 3x3 same-conv.

Conv is computed as 9 shifted matmuls (x2 input-channel chunks) accumulated
in PSUM, bf16 compute / fp32 accumulate. fmap/weights are cast to bf16 on
the host (the kernel computed in bf16 anyway) to halve input DMA; softmax over
the 4 kernel logits is computed on the host; output is written fp32 from
the demod-scaling psum drain.

Row-tile groups of 4: within a group the taps are outer and the row tiles
inner, so the stationary PE weights are reused 4x (deduped Ldweights) while
group drains still overlap the next group's matmuls.
"""

from contextlib import ExitStack

import numpy as np

import concourse.bass as bass
import concourse.mybir as mybir
import concourse.tile as tile
from concourse import bacc
from concourse.bass_utils import run_bass_kernel_spmd

F32 = mybir.dt.float32
BF16 = mybir.dt.bfloat16
NP_BF16 = mybir.dt.np(BF16)

N_CORES = 8
B_LOC = 2          # samples per core
C = 256            # input channels (I)
O = 256            # output channels
H = W = 64
K = 3
NK = 4             # num base kernels
CI = 2             # input channel chunks of 128
CO = 2             # output channel chunks of 128
NT = 8             # row tiles (8 rows x 64 cols = 512 free)
GRP = 4            # row tiles per drain group
ROWS_PER_NT = H // NT


def _build_nc(repeat=1, loop_n=0, parts="full"):
    nc = bacc.Bacc("TRN2", target_bir_lowering=False, debug=False,
                   num_devices=N_CORES)
    fmap = nc.declare_dram_parameter("fmap", [B_LOC, C, H, W], BF16,
                                     isOutput=False)
    mod = nc.declare_dram_parameter("mod", [B_LOC, C], F32, isOutput=False)
    # softmax(kernel_mod) is computed on the host (tiny [B,4] op) so the
    # device pipeline has no ACT dependency before the first weight mix
    kmod = nc.declare_dram_parameter("attn_in", [B_LOC, NK], F32,
                                     isOutput=False)
    # weights are host-permuted to [n, o, ci, ky, kx, i128] so the on-chip
    # mix pipeline is (kl, i)-ordered with packed last dims throughout (DVE
    # 2x mode), the per-tap transpose input needs no strided rearrange, and
    # each per-(n, co, ci) tile is one DMA of contiguous 2.3KB runs
    weights = nc.declare_dram_parameter("weights", [NK, O, CI, K, K, 128],
                                        BF16, isOutput=False)
    out = nc.declare_dram_parameter("out", [B_LOC, O, H, W], F32,
                                    isOutput=True)

    with ExitStack() as ctx:
        tc = ctx.enter_context(tile.TileContext(nc))
        pools = _make_pools(ctx, tc)
        if loop_n:
            with tc.For_i(0, loop_n, 1):
                _build_body(tc, pools, fmap.ap(), mod.ap(), kmod.ap(),
                            weights.ap(), out.ap(), parts, warmup=False)
        else:
            for r in range(repeat):
                _build_body(tc, pools, fmap.ap(), mod.ap(), kmod.ap(),
                            weights.ap(), out.ap(), parts, warmup=(r == 0))
    _dedupe_ldweights(nc)
    nc.compile()
    return nc


def _dedupe_ldweights(nc):
    """Remove PE weight reloads that are byte-identical to the previous
    Ldweights and carry no semaphore waits/updates (the split emits one
    Ldweights per matmul even when the stationary operand is unchanged)."""
    removed = 0
    pe = mybir.EngineType.PE
    for blk in nc.main_func.blocks:
        last_key = None
        keep = []
        for inst in blk.instructions:
            tn = type(inst).__name__
            eng = getattr(inst, "engine", None)
            if tn == "InstLdweights":
                key = repr(inst.ins)
                if (key == last_key and inst.sync_info is None):
                    removed += 1
                    continue
                last_key = key
            elif tn == "InstMatmult":
                pass
            elif eng == pe:
                last_key = None
            keep.append(inst)
        blk.instructions[:] = keep
    return removed


def _make_pools(ctx, tc):
    return {
        "const": ctx.enter_context(tc.tile_pool(name="const", bufs=2)),
        "wnat": ctx.enter_context(tc.tile_pool(name="wnat", bufs=3)),
        "mix": ctx.enter_context(tc.tile_pool(name="mix", bufs=4)),
        "wt": ctx.enter_context(tc.tile_pool(name="wt", bufs=B_LOC * CO)),
        "fm": ctx.enter_context(tc.tile_pool(name="fm", bufs=4)),
        "outp": ctx.enter_context(tc.tile_pool(name="outp", bufs=2)),
        "small": ctx.enter_context(tc.tile_pool(name="small", bufs=12)),
        "psconv": ctx.enter_context(
            tc.tile_pool(name="psconv", bufs=8, space="PSUM")),
    }


def _build_body(tc, pools, fmap, mod, kmod, weights, out, parts="full",
                warmup=True):
    nc = tc.nc

    if parts == "empty":
        t = pools["const"].tile([128, 1], F32, tag="emp")
        nc.vector.memset(t[:], 0.0)
        return

    const = pools["const"]
    wnatp = pools["wnat"]
    mixp = pools["mix"]
    wtp = pools["wt"]
    fmp = pools["fm"]
    outp = pools["outp"]
    smallp = pools["small"]
    psconv = pools["psconv"]

    # ---- small inputs (sync queue) -----------------------------------------
    attn = const.tile([128, B_LOC, NK], F32)
    nc.sync.dma_start(out=attn[:], in_=kmod[None, :, :].broadcast_to(
        [128, B_LOC, NK]))
    # mod lands as one tiny single-partition DMA; broadcast on-chip (the
    # 128-partition broadcast DMA would sit on the startup-critical DMA path)
    m_sm = const.tile([1, B_LOC, C], F32)
    nc.sync.dma_start(out=m_sm[:], in_=mod[None, :, :])
    m_bc = const.tile([128, B_LOC, C], F32)

    eps = const.tile([128, 1], F32)
    nc.vector.memset(eps[:], 1e-8)

    # PE warmup: dummy matmuls during the startup DMA window so the HAM
    # clock gate is at 8/8 (2.4 GHz) when the real conv stream begins (only
    # for the first body; loop iterations are already warm)
    if warmup:
        warm = const.tile([128, 640], BF16)
        nc.vector.memset(warm[:], 0.125)
        wps = psconv.tile([128, GRP, ROWS_PER_NT, W], F32, tag="psg", bufs=2,
                          name="warm_ps")
        wps_flat = wps.rearrange("p g r w -> p (g r w)")
        for i in range(20):
            nc.tensor.matmul(wps_flat[:, 0:512], warm[:, 0:128],
                             warm[:, 128:640], start=(i == 0), stop=(i == 19))

    # preload the Square/Sqrt activation tables off the critical path (the
    # auto-inserted LoadActFuncSet otherwise serializes before the first
    # demod Square)
    preld = const.tile([128, 1], F32)
    nc.scalar.activation(preld[:], eps[:], mybir.ActivationFunctionType.Square)
    nc.scalar.activation(preld[:], eps[:], mybir.ActivationFunctionType.Sqrt,
                         bias=eps[:])
    # the (1 + mod) add is emitted lazily just before its first use so the
    # in-order DVE queue isn't blocked on the m_bc DMA before the first mix
    m_ready = []

    def ensure_m():
        if not m_ready:
            nc.gpsimd.partition_broadcast(m_bc[:], m_sm[0:1, :, :])
            nc.vector.tensor_scalar_add(m_bc[:], m_bc[:], 1.0)  # 1 + mod
            m_ready.append(True)

    # ---- input DMAs: all on the Pool queue, in first-use priority order ----
    # (single queue => transfer order == issue order, so fmaps can't steal
    # DMA bandwidth from the startup-critical weight chunks)
    # w9[co][ci]: [128(o), NK, 9(kl), 128(i)] bf16, one DMA each with
    # contiguous 2.3KB runs
    w9 = [[None] * CI for _ in range(CO)]
    fm_raw = [[None] * CI for _ in range(B_LOC)]

    def load_w(co, ci):
        t = wnatp.tile([128, NK, K * K, 128], BF16, tag=f"wn{co}{ci}",
                       bufs=1, name=f"w9_{co}_{ci}")
        nc.gpsimd.dma_start(
            out=t[:],
            in_=weights[:, co * 128:(co + 1) * 128, ci, :, :, :].rearrange(
                "n o ky kx i -> o n (ky kx) i"))
        w9[co][ci] = t

    def w9_slice(n, co, ci):
        return w9[co][ci][:, n, :, :]

    def load_fmap(b, ci):
        raw = fmp.tile([128, H, W], BF16, tag="fmraw", name=f"fmraw{b}_{ci}")
        nc.gpsimd.dma_start(
            out=raw[:], in_=fmap[b, ci * 128:(ci + 1) * 128, :, :])
        fm_raw[b][ci] = raw

    # NOTE: load emission is interleaved with the weight-pipe blocks below —
    # Tile chains each DMA-family instruction to the completion of the one
    # emitted just before it, so a transpose must not be preceded by a DMA
    # it doesn't actually need.

    # ---- per-sample weight pipeline ----------------------------------------
    # w_T[b][co]: [128(i in chunk), (ci,kl)=18, o-chunk=128] bf16 modulated
    # transposed weights; one xbar transpose per (b, co, ci) half.
    w_T = [[None] * CO for _ in range(B_LOC)]
    dscale = [[None] * CO for _ in range(B_LOC)]
    den_h = [[[None] * CI for _ in range(CO)] for _ in range(B_LOC)]

    wmods = [[[None] * CI for _ in range(CO)] for _ in range(B_LOC)]

    def mix_block(b, co, ci, transposes=True):
        """mix 4 base kernels + modulate by (1+mod) -> wmod (kl, i)-ordered."""
        wn = [w9_slice(n, co, ci) for n in range(NK)]
        t0 = mixp.tile([128, K * K, 128], BF16, tag="mixa")
        t1 = mixp.tile([128, K * K, 128], BF16, tag="mixb")
        nc.vector.tensor_scalar_mul(t0[:], wn[0], attn[:, b, 0:1])
        nc.vector.tensor_scalar_mul(t1[:], wn[1], attn[:, b, 1:2])
        nc.vector.tensor_add(t0[:], t0[:], t1[:])
        nc.vector.tensor_scalar_mul(t1[:], wn[2], attn[:, b, 2:3])
        nc.vector.tensor_add(t0[:], t0[:], t1[:])
        nc.vector.tensor_scalar_mul(t1[:], wn[3], attn[:, b, 3:4])
        nc.vector.tensor_add(t0[:], t0[:], t1[:])
        ensure_m()
        wmod = mixp.tile([128, K * K, 128], BF16, tag="wmod", bufs=4)
        nc.vector.tensor_mul(
            wmod[:], t0[:],
            m_bc[:, b, None, ci * 128:(ci + 1) * 128].broadcast_to(
                [128, K * K, 128]))
        wmods[b][co][ci] = wmod
        # transpose fires as soon as this half's wmod is ready (it gates the
        # conv); the demod scale is applied at psum-drain time instead
        if transposes:
            wt = w_T[b][co]
            nc.sync.dma_start(out=wt[:, ci * K * K:(ci + 1) * K * K, :],
                              in_=wmod[:], transpose=True)
        # demod denominator half: sum over free dims of wmod^2 (per o-part)
        sqscratch = mixp.tile([128, K * K, 128], BF16, tag="sqs", bufs=2)
        dh = smallp.tile([128, 1], F32, tag="den", name=f"den{b}_{co}_{ci}")
        nc.scalar.activation(
            sqscratch[:], wmod[:],
            mybir.ActivationFunctionType.Square, accum_out=dh[:])
        den_h[b][co][ci] = dh

    def finalize_block(b, co):
        ds = smallp.tile([128, 1], F32, tag="dsc")
        nc.vector.tensor_add(ds[:], den_h[b][co][0][:], den_h[b][co][1][:])
        nc.scalar.activation(ds[:], ds[:],
                             mybir.ActivationFunctionType.Sqrt, bias=eps[:])
        nc.vector.reciprocal(ds[:], ds[:])
        dscale[b][co] = ds

    def pipe(b, co, transposes=True):
        wt = wtp.tile([128, K * K * CI, 128], BF16, tag="wt",
                      name=f"wT{b}_{co}")
        w_T[b][co] = wt
        if not transposes:
            nc.vector.memset(wt[:], 0.25)
        for ci in range(CI):
            mix_block(b, co, ci, transposes)
        finalize_block(b, co)

    if parts == "conv":
        for b in range(B_LOC):
            for ci in range(CI):
                load_fmap(b, ci)
            for co in range(CO):
                wt = wtp.tile([128, K * K * CI, 128], BF16, tag="wt",
                              name=f"wTd{b}_{co}")
                nc.vector.memset(wt[:], 0.25)
                w_T[b][co] = wt
                ds = smallp.tile([128, 1], F32, tag="dsc")
                nc.vector.memset(ds[:], 1.0)
                dscale[b][co] = ds
    else:
        tr = parts != "wnotr"
        load_w(0, 0)
        load_fmap(0, 0)
        load_w(0, 1)
        pipe(0, 0, tr)
        load_fmap(0, 1)
        load_w(1, 0)
        load_w(1, 1)
        pipe(0, 1, tr)
        load_fmap(1, 0)
        load_fmap(1, 1)
        pipe(1, 0, tr)
        pipe(1, 1, tr)

    # ---- conv: out[o, y, x] += sum_{ci,ky,kx} w.T @ fmap_shifted -----------
    # Row-tile groups of GRP. Within a group: ci -> tap -> row-tile, so the
    # stationary weights are constant across the GRP inner matmuls (deduped
    # Ldweights) and each group's psum drains overlap the next group.
    # Boundary taps use column/row-clamped access patterns; elements a tap
    # skips are covered by other taps' writes (per-element has_written).
    def conv(b, co):
        for g in range(NT // GRP):
            # one 4-bank psum tile per group; each matmul writes a 1-bank
            # slice; the whole group DMAs straight to DRAM (weights carry
            # the demod scale already)
            ps = psconv.tile([128, GRP, ROWS_PER_NT, W], F32, tag="psg",
                             bufs=2, name=f"ps{b}_{co}_{g}")
            for ci in range(CI):
                for ky in range(K):
                    for kx in range(K):
                        kl = ky * K + kx
                        lhsT = w_T[b][co][:, ci * K * K + kl, :]
                        x0 = max(0, kx - 1)
                        xo = max(0, 1 - kx)
                        wn = W - abs(kx - 1)
                        for j in range(GRP):
                            nt = g * GRP + j
                            r0 = nt * ROWS_PER_NT + ky - 1
                            ny = ROWS_PER_NT
                            rskip = 0
                            if r0 < 0:                # clamp top
                                r0, ny, rskip = 0, ROWS_PER_NT - 1, 1
                            if r0 + ny > H:           # clamp bottom
                                ny = H - r0
                            rhs = fm_raw[b][ci][:, r0:r0 + ny, x0:x0 + wn]
                            nc.tensor.matmul(
                                ps[:, j, rskip:rskip + ny, xo:xo + wn],
                                lhsT, rhs,
                                start=(ci == 0 and kl == 0),
                                stop=(ci == CI - 1 and kl == K * K - 1))
            # drain: demod scale applied here (per-o-partition), fp32 out via
            # the ACT HWDGE queue so the Pool queue stays free for input DMAs
            ot = outp.tile([128, GRP * ROWS_PER_NT, W], F32, tag="ot", bufs=2)
            nc.scalar.mul(ot[:], ps.rearrange("p g r w -> p (g r) w"),
                          dscale[b][co][:])
            nc.scalar.dma_start(
                out=out[b, co * 128:(co + 1) * 128,
                        g * GRP * ROWS_PER_NT:(g + 1) * GRP * ROWS_PER_NT, :],
                in_=ot[:])

    if parts not in ("wpipe", "wnotr"):
        for b in range(B_LOC):
            for co in range(CO):
                conv(b, co)


_NC_CACHE = {}


def _get_nc(repeat=1, loop_n=0, parts="full"):
    key = (repeat, loop_n, parts)
    if key not in _NC_CACHE:
        _NC_CACHE[key] = _build_nc(repeat, loop_n, parts)
    return _NC_CACHE[key]


def _make_in_maps(fmap, mod, kernel_mod, weights):
    fmap_bf = np.ascontiguousarray(fmap.astype(NP_BF16))
    # [n, o, i, ky, kx] -> [n, o, ci, ky, kx, i128] (see DRAM declaration)
    weights_bf = np.ascontiguousarray(
        weights.astype(NP_BF16)
        .reshape(NK, O, CI, 128, K, K)
        .transpose(0, 1, 2, 4, 5, 3))
    # host-side softmax over the 4 kernel logits (tiny)
    e = np.exp(kernel_mod.astype(np.float64)
               - kernel_mod.max(axis=-1, keepdims=True))
    attn = (e / e.sum(axis=-1, keepdims=True)).astype(np.float32)
    in_maps = []
    for c in range(N_CORES):
        s = slice(c * B_LOC, (c + 1) * B_LOC)
        in_maps.append({
            "fmap": np.ascontiguousarray(fmap_bf[s]),
            "mod": np.ascontiguousarray(mod[s]),
            "attn_in": np.ascontiguousarray(attn[s]),
            "weights": weights_bf,
        })
    return in_maps


def kernel(fmap, mod, kernel_mod, weights, _trace=False):
    fmap = np.asarray(fmap, dtype=np.float32)
    mod = np.asarray(mod, dtype=np.float32)
    kernel_mod = np.asarray(kernel_mod, dtype=np.float32)
    weights = np.ascontiguousarray(np.asarray(weights, dtype=np.float32))

    nc = _get_nc()
    in_maps = _make_in_maps(fmap, mod, kernel_mod, weights)
    res = run_bass_kernel_spmd(nc, in_maps, list(range(N_CORES)), trace=_trace)
    outs = np.concatenate(
        [res.results[c]["out"].astype(np.float32) for c in range(N_CORES)],
        axis=0)
    if _trace:
        kernel.last_results = res
    return outs
